# revision 48
# baseline (speedup 1.0000x reference)
"""Trainium2 Bass kernel for nn_MiningGNN (2-layer GAT message passing).

Sharding: nodes range-sharded across 8 cores; edges sharded by destination
owner. Within a core, edges are grouped into 32-node dst buckets padded to a
uniform tile capacity, so a single SPMD program serves all cores. Per conv
layer each core computes its shard of the node table
[x1(32 bf16) | a_src | 1.0 | a_dst | pad] (18 f32 rows), all-gathers it, then
streams its edges: ONE batched indirect-DMA per chunk gathers x1/a_src rows
for 128*CT edges (amortizes the ~1us SWDGE fixed cost that dominated the
per-tile-gather baseline), builds attention coefficients (the per-dst term is
expanded through a 32-wide one-hot), scales the one-hot by exp(z) and
scatter-accumulates numerator+denominator per dst node with one-hot matmuls
on the TensorEngine (the constant-1 table column yields the denominator).
Per-edge ea = attr @ (We @ att_e) and per-node easum/deg (graph-static
reductions) are precomputed on the host. Node-major epilogue applies the
self-loop (fill_value='mean' via easum/deg), normalization, bias and
activation. segment_max is skipped (softmax is shift-invariant; scores are
tiny and clamped).
"""
import numpy as np

P = 128          # partitions / edge-tile height
B = 32           # dst-bucket width (nodes)
ROW = 48         # stag row elems (bf16): [x1(32)|a_src|a_dst|one|pad]
CG = 10          # buckets per edge-pipeline chunk
NCA = 448        # f-major node chunk (divides nloc)
OOB = 1 << 28    # src index for padding slots (bounds-checked, skipped)
NEG = 0.2        # leaky_relu slope
ZCLAMP = 30.0
NR = 35          # scatter cols: [num(32) | junk | junk | denom]


def _snake(ids_sorted, n_bins):
    """Deal ids (desc-degree order) into n_bins boustrophedon; -1 pads."""
    n = len(ids_sorted)
    nrows = -(-n // n_bins)
    padded = np.full(nrows * n_bins, -1, np.int64)
    padded[:n] = ids_sorted
    g = padded.reshape(nrows, n_bins)
    g[1::2] = g[1::2, ::-1]
    return g


# ----------------------------------------------------------------- host layout
def _build_layout(src, dst, ea1, ea2, n_nodes, n_cores):
    """Relabel nodes degree-balanced across cores/buckets, sort edges by new
    dst, bucket and pad to uniform tiles. Returns newidx (orig -> new id)."""
    from math import lcm
    deg_i = np.bincount(dst, minlength=n_nodes)
    nloc_raw = -(-n_nodes // n_cores)
    L = lcm(CG, NCA // B, 4)
    tgt = max(-(-nloc_raw // B),
              int(len(dst) / n_cores * 1.03 / 1000))     # buckets ~<=1000 edges
    nbkt = -(-tgt // L) * L
    nloc = nbkt * B
    n_pad = nloc * n_cores

    # degree-balanced node -> (core, bucket, slot) via snake dealing
    order_deg = np.argsort(-deg_i, kind="stable")
    gcore = _snake(order_deg, n_cores)                   # [rows, n_cores]
    newidx = np.empty(n_nodes, np.int64)
    for c in range(n_cores):
        nodes_c = gcore[:, c]
        nodes_c = nodes_c[nodes_c >= 0]
        nodes_c = nodes_c[np.argsort(-deg_i[nodes_c], kind="stable")]
        gb = _snake(nodes_c, nbkt)                       # [rows, nbkt]
        r_idx, b_idx = np.nonzero(gb >= 0)
        newidx[gb[r_idx, b_idx]] = c * nloc + b_idx * B + r_idx

    ndst = newidx[dst]
    nsrc = newidx[src].astype(np.int32)

    deg = np.zeros(n_pad, np.float32)
    es1 = np.zeros(n_pad, np.float32)
    es2 = np.zeros(n_pad, np.float32)
    deg[:] = np.bincount(ndst, minlength=n_pad)
    es1[:] = np.bincount(ndst, weights=ea1, minlength=n_pad)
    es2[:] = np.bincount(ndst, weights=ea2, minlength=n_pad)

    order = np.argsort(ndst, kind="stable")
    s_s = nsrc[order]
    d_s = ndst[order]
    e1_s = ea1[order].astype(np.float32)
    e2_s = ea2[order].astype(np.float32)
    core_of = d_s // nloc
    bkt_loc = (d_s % nloc) // B
    counts = np.bincount(core_of * nbkt + bkt_loc, minlength=nbkt * n_cores)
    cap = int(-(-counts.max() // P))
    tt = nbkt * cap

    starts = np.zeros(nbkt * n_cores + 1, np.int64)
    np.cumsum(counts, out=starts[1:])
    rank = np.arange(len(d_s), dtype=np.int64) - starts[core_of * nbkt + bkt_loc]
    slot = bkt_loc * (cap * P) + rank
    lane = slot % P
    tile = slot // P

    src_t = np.full((n_cores, P, tt), OOB, np.int32)
    dstl_t = np.full((n_cores, P, tt), OOB, np.int32)
    dl_t = np.full((n_cores, P, tt), -1.0, np.float32)
    e1_t = np.zeros((n_cores, P, tt), np.float32)
    e2_t = np.zeros((n_cores, P, tt), np.float32)
    src_t[core_of, lane, tile] = s_s
    dstl_t[core_of, lane, tile] = (d_s % nloc).astype(np.int32)
    dl_t[core_of, lane, tile] = (d_s % B).astype(np.float32)
    e1_t[core_of, lane, tile] = e1_s
    e2_t[core_of, lane, tile] = e2_s

    nch = nloc // P
    he = np.stack([es1, es2, deg], axis=-1).reshape(n_cores, nch, P, 3)
    he_t = np.ascontiguousarray(he.transpose(0, 2, 1, 3))  # [cores, P, nch, 3]
    cfg = dict(nloc=nloc, nbkt=nbkt, cap=cap, tt=tt, n_pad=n_pad,
               n_cores=n_cores)
    return cfg, src_t, dstl_t, dl_t, e1_t, e2_t, he_t, newidx


# ------------------------------------------------------------- device program
def _build_program(cfg):
    import concourse.bass as bass
    import concourse.mybir as mybir
    import concourse.tile as tile
    from concourse import bacc
    from concourse.masks import make_identity
    from contextlib import ExitStack

    f32 = mybir.dt.float32
    bf16 = mybir.dt.bfloat16
    i32 = mybir.dt.int32
    AT = mybir.AluOpType
    AF = mybir.ActivationFunctionType
    AX = mybir.AxisListType

    nloc, nbkt, cap, tt = cfg["nloc"], cfg["nbkt"], cfg["cap"], cfg["tt"]
    n_pad, n_cores = cfg["n_pad"], cfg["n_cores"]
    nchunk = nbkt // CG
    CT = CG * cap                    # edge tiles per chunk
    CN = CG * B                      # nodes per chunk (256)
    NCH = nloc // P                  # node-major chunks
    NC_A = nloc // NCA               # f-major chunks
    groups = [list(range(n_cores))]

    nc = bacc.Bacc("TRN2", target_bir_lowering=False, debug=False,
                   num_devices=n_cores)

    # ---------------- external inputs
    xT = nc.dram_tensor("xT", [5, nloc], f32, kind="ExternalInput")
    src_d = nc.dram_tensor("src", [P, tt], i32, kind="ExternalInput")
    dstl_d = nc.dram_tensor("dstl", [P, tt], i32, kind="ExternalInput")
    dl_d = nc.dram_tensor("dl", [P, tt], bf16, kind="ExternalInput")
    ea1_d = nc.dram_tensor("ea1", [P, tt], f32, kind="ExternalInput")
    ea2_d = nc.dram_tensor("ea2", [P, tt], f32, kind="ExternalInput")
    he_d = nc.dram_tensor("he", [P, NCH, 3], f32, kind="ExternalInput")
    wnames = [("enc_W", [5, 32]), ("enc_b", [32, 1]),
              ("c1_W", [32, 32]), ("c1_asad", [32, 2]), ("c1_b", [1, 32]),
              ("c2_W", [32, 32]), ("c2_asad", [32, 2]), ("c2_b", [1, 32]),
              ("dec_WT", [1, 128]), ("dec_b", [1, 4])]
    wh = {n: nc.dram_tensor(n, s, f32, kind="ExternalInput")
          for n, s in wnames}
    out_d = nc.dram_tensor("out", [nloc, 4], f32, kind="ExternalOutput")

    # ---------------- internal DRAM
    tab_own = [nc.dram_tensor(f"tab_own{i}", [nloc, 18], f32)
               for i in range(2)]
    tab_full = [nc.dram_tensor(f"tab_full{i}", [n_pad, 18], f32,
                               addr_space="Shared") for i in range(2)]
    num_d = [nc.dram_tensor(f"num{i}", [nloc, NR], f32) for i in range(2)]
    adT_d = [nc.dram_tensor(f"adT{i}", [1, nloc], f32) for i in range(2)]

    with tile.TileContext(nc) as tc, ExitStack() as ctx:
        const = ctx.enter_context(tc.tile_pool(name="const", bufs=1))
        keep = ctx.enter_context(tc.tile_pool(name="keep", bufs=1))
        nodef = ctx.enter_context(tc.tile_pool(name="nodef", bufs=2))
        psn = ctx.enter_context(tc.tile_pool(name="psn", bufs=2,
                                             space="PSUM"))
        psb = ctx.enter_context(tc.tile_pool(name="psb", bufs=2,
                                             space="PSUM"))
        pse = ctx.enter_context(tc.tile_pool(name="pse", bufs=2,
                                             space="PSUM"))
        edge = ctx.enter_context(tc.tile_pool(name="edge", bufs=4))
        nph = ctx.enter_context(tc.tile_pool(name="nph", bufs=1))

        # ---------- constants
        iota_i = const.tile([P, B], i32)
        nc.gpsimd.iota(iota_i[:], pattern=[[1, B]], base=0,
                       channel_multiplier=0)
        iota16 = const.tile([P, B], bf16)
        nc.vector.tensor_copy(iota16[:], iota_i[:])
        ones_row = const.tile([1, P], f32)
        nc.vector.memset(ones_row[:], 1.0)
        ones16 = const.tile([1, P], bf16)
        nc.vector.memset(ones16[:], 1.0)
        ident = const.tile([P, P], bf16)
        make_identity(nc, ident[:])

        sbw = {}
        for n, s in wnames:
            t = const.tile(s, f32, tag=f"w_{n}")
            nc.sync.dma_start(t[:], wh[n][:])
            sbw[n] = t

        he_sb = keep.tile([P, NCH, 3], f32, tag="he")
        nc.sync.dma_start(he_sb[:], he_d[:])

        def bcast_row(row_ap, n, out_dt, pool, tag):
            """[1, n] row -> [P, n] tile via PE outer product."""
            ps = psb.tile([P, n], f32, tag="psb")
            ones = ones16 if row_ap.dtype == bf16 else ones_row
            nc.tensor.matmul(ps[:], lhsT=ones[:], rhs=row_ap,
                             start=True, stop=True)
            out = pool.tile([P, n], out_dt, tag=tag)
            nc.scalar.copy(out[:], ps[:])
            return out

        b_bc = [bcast_row(sbw["c1_b"][:], 32, f32, const, "bbc0"),
                bcast_row(sbw["c2_b"][:], 32, f32, const, "bbc1")]
        wdall = bcast_row(sbw["dec_WT"][:], 128, f32, const, "wdbc")
        wd_bc = [wdall[:, 32 * k:32 * (k + 1)] for k in range(4)]
        bd_bc = bcast_row(sbw["dec_b"][:], 4, f32, const, "bdbc")

        sbw16 = {}
        for n in ("c1_W", "c1_asad", "c2_W", "c2_asad"):
            t16 = const.tile(sbw[n].shape, bf16, tag=f"w16_{n}")
            nc.vector.tensor_copy(t16[:], sbw[n][:])
            sbw16[n] = t16

        # ---------- stage A: encoder (f-major)
        h0T = keep.tile([32, nloc], bf16, tag="hT")
        for c in range(NC_A):
            sl = slice(c * NCA, (c + 1) * NCA)
            xc = nodef.tile([5, NCA], f32, tag="xc")
            nc.sync.dma_start(xc[:], xT[:, sl])
            ps = psn.tile([P, NCA], f32, tag="psn")
            nc.tensor.matmul(ps[0:32, :NCA], lhsT=sbw["enc_W"][:],
                             rhs=xc[:], start=True, stop=True)
            nc.scalar.activation(h0T[:, sl], ps[0:32, :NCA], AF.Relu,
                                 bias=sbw["enc_b"][:], scale=1.0)

        def make_table(hT, wk, asadk, conv):
            """f-major hidden [32, nloc] -> aug rows + node-major stag."""
            augT = keep.tile([ROW, nloc], bf16, tag="augT")
            nc.vector.memset(augT[:], 0.0)
            for c in range(NC_A):
                sl = slice(c * NCA, (c + 1) * NCA)
                ps = psn.tile([P, NCA], f32, tag="psn")
                nc.tensor.matmul(ps[0:32, :NCA], lhsT=sbw16[wk][:],
                                 rhs=hT[:, sl], start=True, stop=True)
                nc.scalar.copy(augT[0:32, sl], ps[0:32, :NCA])
                ps1 = psn.tile([P, NCA], f32, tag="psn")
                nc.tensor.matmul(ps1[0:2, :NCA], lhsT=sbw16[asadk][:],
                                 rhs=augT[0:32, sl], start=True, stop=True)
                nc.vector.tensor_copy(augT[32:34, sl], ps1[0:2, :NCA])
            nc.gpsimd.dma_start(adT_d[conv][:], augT[33:34, :])
            stag = keep.tile([P, NCH, ROW], bf16, tag="stag")
            for c in range(NCH):
                ps = psn.tile([P, P], bf16, tag="psnT")
                nc.tensor.transpose(out=ps[:, 0:ROW],
                                    in_=augT[:, c * P:(c + 1) * P],
                                    identity=ident[0:ROW, 0:ROW])
                nc.scalar.copy(stag[:, c, :], ps[:, 0:ROW])
            nc.vector.memset(stag[:, :, 34], 1.0)
            return stag

        def publish_table(stag, conv):
            own_view = tab_own[conv][:].rearrange("(c p) r -> p c r", p=P)
            nc.sync.dma_start(own_view, stag[:, :, 0:36].bitcast(f32))
            nc.gpsimd.collective_compute(
                "AllGather", mybir.AluOpType.bypass,
                replica_groups=groups,
                ins=[tab_own[conv][:]],
                outs=[tab_full[conv][:]],
            )

        stag1 = make_table(h0T, "c1_W", "c1_asad", 0)
        publish_table(stag1, 0)

        # ---------- edge pipeline
        def edge_pass(conv):
            ea_d = ea1_d if conv == 0 else ea2_d
            for ch in range(nchunk):
                tsl = slice(ch * CT, (ch + 1) * CT)
                srcs = edge.tile([P, CT], i32, tag="srcs")
                nc.sync.dma_start(srcs[:], src_d[:, tsl])
                dls = edge.tile([P, CT], bf16, tag="dls")
                nc.sync.dma_start(dls[:], dl_d[:, tsl])
                ea = edge.tile([P, CT], f32, tag="ea")
                nc.sync.dma_start(ea[:], ea_d[:, tsl])
                vp = edge.tile([P, CT, 18], f32, tag="v")
                v = vp[:].bitcast(bf16)          # [P, CT, 36]
                if conv == 0 and ch < 4:         # one memset per pool buffer
                    nc.vector.memset(vp[:], 0.0)
                nc.gpsimd.indirect_dma_start(
                    out=vp[:], out_offset=None,
                    in_=tab_full[conv][:],
                    in_offset=bass.IndirectOffsetOnAxis(ap=srcs[:], axis=0),
                    bounds_check=n_pad - 1, oob_is_err=False)
                oh = edge.tile([P, CT, B], bf16, tag="oh")
                nc.vector.tensor_tensor(
                    out=oh[:], in0=dls[:].to_broadcast([P, CT, B]),
                    in1=iota16[:, None, :].to_broadcast([P, CT, B]),
                    op=AT.is_equal)
                dstls = edge.tile([P, CT], i32, tag="dstls")
                nc.sync.dma_start(dstls[:], dstl_d[:, tsl])
                z = edge.tile([P, CT], f32, tag="z")
                nc.vector.tensor_copy(z[:], v[:, :, 32])   # a_src[src]
                nc.vector.tensor_tensor(out=z[:], in0=z[:], in1=ea[:],
                                        op=AT.add)
                # z += a_dst[dst] via 4B gather-accumulate on gpsimd
                nc.gpsimd.indirect_dma_start(
                    out=z[:], out_offset=None,
                    in_=adT_d[conv][:],
                    in_offset=bass.IndirectOffsetOnAxis(ap=dstls[:], axis=1),
                    bounds_check=nloc - 1, oob_is_err=False,
                    compute_op=AT.add)
                nc.vector.tensor_scalar_min(z[:], z[:], ZCLAMP)
                zn = edge.tile([P, CT], f32, tag="zn")
                nc.vector.tensor_scalar_mul(zn[:], z[:], NEG)
                nc.vector.tensor_tensor(out=z[:], in0=z[:], in1=zn[:],
                                        op=AT.max)
                ex = edge.tile([P, CT], bf16, tag="ex")
                nc.scalar.activation(ex[:], z[:], AF.Exp)
                ohex = edge.tile([P, CT, B], bf16, tag="ohex")
                nc.vector.tensor_tensor(
                    out=ohex[:], in0=oh[:],
                    in1=ex[:, :, None].to_broadcast([P, CT, B]),
                    op=AT.mult)
                ps = pse.tile([B, CG * NR], f32, tag="pse")
                for g in range(CG):
                    for i in range(cap):
                        t = g * cap + i
                        nc.tensor.matmul(
                            ps[:, g * NR:(g + 1) * NR],
                            lhsT=ohex[:, t, :], rhs=v[:, t, 0:NR],
                            start=(i == 0), stop=(i == cap - 1))
                st = edge.tile([B, CG * NR], f32, tag="st")
                nc.scalar.copy(st[:], ps[:])
                nc.sync.dma_start(
                    num_d[conv][ch * CN:(ch + 1) * CN]
                    .rearrange("(g b) r -> b g r", b=B),
                    st[:].rearrange("b (g r) -> b g r", r=NR))

        edge_pass(0)

        # ---------- node phase (sliced per pair of edge chunks so it
        # overlaps the edge-pass tail; 2*CN nodes = NSL node-chunks)
        NSL = 2 * CN // P

        def node_finish(conv, stag, out_relu):
            h = nph.tile([P, NCH, 32], f32, tag="h")
            for s in range(nchunk // 2):
                csl = slice(s * NSL, (s + 1) * NSL)
                num = nodef.tile([P, NSL, NR], f32, tag="num")
                nc.sync.dma_start(
                    num[:],
                    num_d[conv][s * NSL * P:(s + 1) * NSL * P]
                    .rearrange("(c p) r -> p c r", p=P))
                dg = nodef.tile([P, NSL], f32, tag="dg")
                nc.vector.tensor_scalar_max(dg[:], he_sb[:, csl, 2], 1.0)
                nc.vector.reciprocal(dg[:], dg[:])
                zl = nodef.tile([P, NSL], f32, tag="zl2")
                nc.vector.tensor_tensor(out=zl[:], in0=he_sb[:, csl, conv],
                                        in1=dg[:], op=AT.mult)
                asf = nodef.tile([P, NSL], f32, tag="asf")
                nc.vector.tensor_copy(asf[:], stag[:, csl, 32])
                nc.vector.tensor_tensor(out=zl[:], in0=zl[:], in1=asf[:],
                                        op=AT.add)
                nc.vector.tensor_copy(asf[:], stag[:, csl, 33])
                nc.vector.tensor_tensor(out=zl[:], in0=zl[:], in1=asf[:],
                                        op=AT.add)
                zln = nodef.tile([P, NSL], f32, tag="zln")
                nc.vector.tensor_scalar_mul(zln[:], zl[:], NEG)
                nc.vector.tensor_tensor(out=zl[:], in0=zl[:], in1=zln[:],
                                        op=AT.max)
                exl = nodef.tile([P, NSL], f32, tag="exl")
                nc.scalar.activation(exl[:], zl[:], AF.Exp)
                den = nodef.tile([P, NSL], f32, tag="den")
                nc.vector.tensor_tensor(out=den[:], in0=num[:, :, 34],
                                        in1=exl[:], op=AT.add)
                nc.vector.reciprocal(den[:], den[:])
                exl16 = nodef.tile([P, NSL], bf16, tag="exl16")
                nc.vector.tensor_copy(exl16[:], exl[:])
                hs = h[:, csl, :]
                nc.vector.tensor_tensor(
                    out=hs, in0=stag[:, csl, 0:32],
                    in1=exl16[:, :, None].to_broadcast([P, NSL, 32]),
                    op=AT.mult)
                nc.vector.tensor_tensor(out=hs, in0=hs,
                                        in1=num[:, :, 0:32], op=AT.add)
                nc.vector.tensor_tensor(
                    out=hs, in0=hs,
                    in1=den[:, :, None].to_broadcast([P, NSL, 32]),
                    op=AT.mult)
                nc.vector.tensor_tensor(
                    out=hs, in0=hs,
                    in1=b_bc[conv][:, None, :].to_broadcast([P, NSL, 32]),
                    op=AT.add)
                if out_relu:
                    nc.vector.tensor_scalar_max(hs, hs, 0.0)
            return h

        h1 = node_finish(0, stag1, True)

        # node-major -> f-major via PE transpose chunks
        h1T = keep.tile([32, nloc], bf16, tag="hT")
        h1b = nph.tile([P, NCH, 32], bf16, tag="h1b")
        for s in range(nchunk // 2):
            csl = slice(s * NSL, (s + 1) * NSL)
            nc.vector.tensor_copy(h1b[:, csl, :], h1[:, csl, :])
        for c in range(NCH):
            ps = psn.tile([P, P], bf16, tag="psnT")
            nc.tensor.transpose(out=ps[0:32, 0:P], in_=h1b[:, c, :],
                                identity=ident[:])
            nc.scalar.copy(h1T[:, c * P:(c + 1) * P], ps[0:32, 0:P])

        stag2 = make_table(h1T, "c2_W", "c2_asad", 1)
        publish_table(stag2, 1)
        edge_pass(1)
        h2 = node_finish(1, stag2, False)

        # ---------- decoder + log_softmax (node-major, sliced to cascade
        # behind node_finish(1) slices)
        for s in range(nchunk // 2):
            csl = slice(s * NSL, (s + 1) * NSL)
            lg = nodef.tile([P, NSL, 4], f32, tag="lg")
            tmp = nodef.tile([P, NSL, 32], f32, tag="dtmp")
            for k in range(4):
                nc.vector.tensor_tensor(
                    out=tmp[:], in0=h2[:, csl, :],
                    in1=wd_bc[k][:, None, :].to_broadcast([P, NSL, 32]),
                    op=AT.mult)
                nc.vector.tensor_reduce(out=lg[:, :, k], in_=tmp[:],
                                        axis=AX.X, op=AT.add)
            nc.vector.tensor_tensor(
                out=lg[:], in0=lg[:],
                in1=bd_bc[:, None, 0:4].to_broadcast([P, NSL, 4]), op=AT.add)
            mx = nodef.tile([P, NSL], f32, tag="mx")
            nc.vector.tensor_reduce(out=mx[:], in_=lg[:], axis=AX.X,
                                    op=AT.max)
            nc.vector.tensor_tensor(
                out=lg[:], in0=lg[:],
                in1=mx[:, :, None].to_broadcast([P, NSL, 4]),
                op=AT.subtract)
            el = nodef.tile([P, NSL, 4], f32, tag="el")
            nc.scalar.activation(el[:], lg[:], AF.Exp)
            se = nodef.tile([P, NSL], f32, tag="se")
            nc.vector.tensor_reduce(out=se[:], in_=el[:], axis=AX.X,
                                    op=AT.add)
            ls = nodef.tile([P, NSL], f32, tag="ls")
            nc.scalar.activation(ls[:], se[:], AF.Ln)
            nc.vector.tensor_tensor(
                out=lg[:], in0=lg[:],
                in1=ls[:, :, None].to_broadcast([P, NSL, 4]),
                op=AT.subtract)
            nc.sync.dma_start(
                out_d[s * NSL * P:(s + 1) * NSL * P]
                .rearrange("(c p) r -> p c r", p=P), lg[:])

    nc.compile()
    return nc


_PROGRAM_CACHE = {}


def _get_program(cfg):
    key = (cfg["nloc"], cfg["cap"])
    if key not in _PROGRAM_CACHE:
        _PROGRAM_CACHE[key] = _build_program(cfg)
    return _PROGRAM_CACHE[key]


def _prep(inputs):
    """Host preprocessing: layout + per-core input maps."""
    import ml_dtypes
    f32 = np.float32
    ei = np.asarray(inputs["edge_index"])
    attr = np.asarray(inputs["edge_attr"], f32)
    n_nodes = np.asarray(inputs["x"]).shape[0]
    n_cores = 8
    src = ei[0].astype(np.int32)
    dst = ei[1].astype(np.int32)

    w1 = (np.asarray(inputs["c1_We"], f32)
          @ np.asarray(inputs["c1_att_e"], f32))
    w2 = (np.asarray(inputs["c2_We"], f32)
          @ np.asarray(inputs["c2_att_e"], f32))
    ea1 = (attr @ w1).astype(f32)
    ea2 = (attr @ w2).astype(f32)

    cfg, src_t, dstl_t, dl_t, e1_t, e2_t, he_t, newidx = _build_layout(
        src, dst, ea1, ea2, n_nodes, n_cores)

    x = np.asarray(inputs["x"], f32)
    nloc, n_pad = cfg["nloc"], cfg["n_pad"]
    xp = np.zeros((n_pad, 5), f32)
    xp[newidx] = x
    cfg["newidx"] = newidx
    com = {
        "enc_W": np.asarray(inputs["enc_W"], f32),
        "enc_b": np.asarray(inputs["enc_b"], f32).reshape(32, 1),
        "c1_W": np.asarray(inputs["c1_W"], f32),
        "c1_asad": np.stack([np.asarray(inputs["c1_att_src"], f32),
                             np.asarray(inputs["c1_att_dst"], f32)], axis=1),
        "c1_b": np.asarray(inputs["c1_b"], f32).reshape(1, 32),
        "c2_W": np.asarray(inputs["c2_W"], f32),
        "c2_asad": np.stack([np.asarray(inputs["c2_att_src"], f32),
                             np.asarray(inputs["c2_att_dst"], f32)], axis=1),
        "c2_b": np.asarray(inputs["c2_b"], f32).reshape(1, 32),
        "dec_WT": np.asarray(inputs["dec_W"], f32).T.copy().reshape(1, 128),
        "dec_b": np.asarray(inputs["dec_b"], f32).reshape(1, 4),
    }
    in_maps = []
    for c in range(n_cores):
        m = dict(com)
        m["xT"] = xp[c * nloc:(c + 1) * nloc].T.copy()
        m["src"] = src_t[c]
        m["dstl"] = dstl_t[c]
        m["dl"] = dl_t[c].astype(ml_dtypes.bfloat16)
        m["ea1"] = e1_t[c]
        m["ea2"] = e2_t[c]
        m["he"] = he_t[c]
        in_maps.append(m)
    return cfg, in_maps


# ------------------------------------------------------------------ entrypoint
def kernel(**inputs):
    n_trucks = int(inputs["num_trucks"])
    cfg, in_maps = _prep(inputs)
    n_cores = cfg["n_cores"]

    nc = _get_program(cfg)
    from concourse.bass_utils import run_bass_kernel_spmd
    res = run_bass_kernel_spmd(nc, in_maps, core_ids=list(range(n_cores)),
                               trace=False)
    outs = [res.results[c]["out"] for c in range(n_cores)]
    full = np.concatenate(outs, axis=0)[cfg["newidx"][:n_trucks]]
    return np.asarray(full, np.float32)


# revision 56
# speedup vs baseline: 1.0711x; 1.0711x over previous
"""Trainium2 Bass kernel for nn_MiningGNN (2-layer GAT message passing).

Sharding: nodes range-sharded across 8 cores; edges sharded by destination
owner. Within a core, edges are grouped into 32-node dst buckets padded to a
uniform tile capacity, so a single SPMD program serves all cores. Per conv
layer each core computes its shard of the node table
[x1(32 bf16) | a_src | 1.0 | a_dst | pad] (18 f32 rows), all-gathers it, then
streams its edges: ONE batched indirect-DMA per chunk gathers x1/a_src rows
for 128*CT edges (amortizes the ~1us SWDGE fixed cost that dominated the
per-tile-gather baseline), builds attention coefficients (the per-dst term is
expanded through a 32-wide one-hot), scales the one-hot by exp(z) and
scatter-accumulates numerator+denominator per dst node with one-hot matmuls
on the TensorEngine (the constant-1 table column yields the denominator).
Per-edge ea = attr @ (We @ att_e) and per-node easum/deg (graph-static
reductions) are precomputed on the host. Node-major epilogue applies the
self-loop (fill_value='mean' via easum/deg), normalization, bias and
activation. segment_max is skipped (softmax is shift-invariant; scores are
tiny and clamped).
"""
import numpy as np

P = 128          # partitions / edge-tile height
B = 32           # dst-bucket width (nodes)
ROW = 48         # stag row elems (bf16): [x1(32)|a_src|a_dst|one|pad]
CG = 10          # buckets per edge-pipeline chunk
NCA = 448        # f-major node chunk (divides nloc)
OOB = 1 << 28    # src index for padding slots (bounds-checked, skipped)
NEG = 0.2        # leaky_relu slope
ZCLAMP = 30.0
NR = 35          # scatter cols: [num(32) | junk | junk | denom]


def _snake(ids_sorted, n_bins):
    """Deal ids (desc-degree order) into n_bins boustrophedon; -1 pads."""
    n = len(ids_sorted)
    nrows = -(-n // n_bins)
    padded = np.full(nrows * n_bins, -1, np.int64)
    padded[:n] = ids_sorted
    g = padded.reshape(nrows, n_bins)
    g[1::2] = g[1::2, ::-1]
    return g


# ----------------------------------------------------------------- host layout
def _build_layout(src, dst, ea1, ea2, n_nodes, n_cores):
    """Relabel nodes degree-balanced across cores/buckets, sort edges by new
    dst, bucket and pad to uniform tiles. Returns newidx (orig -> new id)."""
    from math import lcm
    deg_i = np.bincount(dst, minlength=n_nodes)
    nloc_raw = -(-n_nodes // n_cores)
    L = lcm(CG, NCA // B, 4)
    tgt = max(-(-nloc_raw // B),
              int(len(dst) / n_cores * 1.03 / 1000))     # buckets ~<=1000 edges
    nbkt = -(-tgt // L) * L
    nloc = nbkt * B
    n_pad = nloc * n_cores

    # degree-balanced node -> (core, bucket, slot) via snake dealing
    order_deg = np.argsort(-deg_i, kind="stable")
    gcore = _snake(order_deg, n_cores)                   # [rows, n_cores]
    newidx = np.empty(n_nodes, np.int64)
    for c in range(n_cores):
        nodes_c = gcore[:, c]
        nodes_c = nodes_c[nodes_c >= 0]
        nodes_c = nodes_c[np.argsort(-deg_i[nodes_c], kind="stable")]
        gb = _snake(nodes_c, nbkt)                       # [rows, nbkt]
        r_idx, b_idx = np.nonzero(gb >= 0)
        newidx[gb[r_idx, b_idx]] = c * nloc + b_idx * B + r_idx

    ndst = newidx[dst]
    nsrc = newidx[src].astype(np.int32)

    deg = np.zeros(n_pad, np.float32)
    es1 = np.zeros(n_pad, np.float32)
    es2 = np.zeros(n_pad, np.float32)
    deg[:] = np.bincount(ndst, minlength=n_pad)
    es1[:] = np.bincount(ndst, weights=ea1, minlength=n_pad)
    es2[:] = np.bincount(ndst, weights=ea2, minlength=n_pad)

    order = np.argsort(ndst, kind="stable")
    s_s = nsrc[order]
    d_s = ndst[order]
    e1_s = ea1[order].astype(np.float32)
    e2_s = ea2[order].astype(np.float32)
    core_of = d_s // nloc
    bkt_loc = (d_s % nloc) // B
    counts = np.bincount(core_of * nbkt + bkt_loc, minlength=nbkt * n_cores)
    cap = int(-(-counts.max() // P))
    tt = nbkt * cap

    starts = np.zeros(nbkt * n_cores + 1, np.int64)
    np.cumsum(counts, out=starts[1:])
    rank = np.arange(len(d_s), dtype=np.int64) - starts[core_of * nbkt + bkt_loc]
    slot = bkt_loc * (cap * P) + rank
    lane = slot % P
    tile = slot // P

    src_t = np.full((n_cores, P, tt), OOB, np.int32)
    dstl_t = np.full((n_cores, P, tt), OOB, np.int32)
    dl_t = np.full((n_cores, P, tt), -1.0, np.float32)
    e1_t = np.zeros((n_cores, P, tt), np.float32)
    e2_t = np.zeros((n_cores, P, tt), np.float32)
    src_t[core_of, lane, tile] = s_s
    dstl_t[core_of, lane, tile] = (d_s % nloc).astype(np.int32)
    dl_t[core_of, lane, tile] = (d_s % B).astype(np.float32)
    e1_t[core_of, lane, tile] = e1_s
    e2_t[core_of, lane, tile] = e2_s

    nch = nloc // P
    he = np.stack([es1, es2, deg], axis=-1).reshape(n_cores, nch, P, 3)
    he_t = np.ascontiguousarray(he.transpose(0, 2, 1, 3))  # [cores, P, nch, 3]
    cfg = dict(nloc=nloc, nbkt=nbkt, cap=cap, tt=tt, n_pad=n_pad,
               n_cores=n_cores)
    return cfg, src_t, dstl_t, dl_t, e1_t, e2_t, he_t, newidx


# ------------------------------------------------------------- device program
def _build_program(cfg):
    import concourse.bass as bass
    import concourse.mybir as mybir
    import concourse.tile as tile
    from concourse import bacc
    from concourse.masks import make_identity
    from contextlib import ExitStack

    f32 = mybir.dt.float32
    bf16 = mybir.dt.bfloat16
    i32 = mybir.dt.int32
    AT = mybir.AluOpType
    AF = mybir.ActivationFunctionType
    AX = mybir.AxisListType

    nloc, nbkt, cap, tt = cfg["nloc"], cfg["nbkt"], cfg["cap"], cfg["tt"]
    n_pad, n_cores = cfg["n_pad"], cfg["n_cores"]
    nchunk = nbkt // CG
    CT = CG * cap                    # edge tiles per chunk
    CN = CG * B                      # nodes per chunk (256)
    NCH = nloc // P                  # node-major chunks
    NC_A = nloc // NCA               # f-major chunks
    groups = [list(range(n_cores))]

    nc = bacc.Bacc("TRN2", target_bir_lowering=False, debug=False,
                   num_devices=n_cores)

    # ---------------- external inputs
    xT = nc.dram_tensor("xT", [5, nloc], f32, kind="ExternalInput")
    src_d = nc.dram_tensor("src", [P, tt], i32, kind="ExternalInput")
    dstl_d = nc.dram_tensor("dstl", [P, tt], i32, kind="ExternalInput")
    dl_d = nc.dram_tensor("dl", [P, tt], bf16, kind="ExternalInput")
    ea1_d = nc.dram_tensor("ea1", [P, tt], f32, kind="ExternalInput")
    ea2_d = nc.dram_tensor("ea2", [P, tt], f32, kind="ExternalInput")
    he_d = nc.dram_tensor("he", [P, NCH, 3], f32, kind="ExternalInput")
    wnames = [("enc_W", [5, 32]), ("enc_b", [32, 1]),
              ("c1_W", [32, 32]), ("c1_asad", [32, 2]), ("c1_b", [1, 32]),
              ("c2_W", [32, 32]), ("c2_asad", [32, 2]), ("c2_b", [1, 32]),
              ("dec_WT", [1, 128]), ("dec_b", [1, 4]),
              ("c1_adr", [1, 32]), ("c2_adr", [1, 32])]
    wh = {n: nc.dram_tensor(n, s, f32, kind="ExternalInput")
          for n, s in wnames}
    out_d = nc.dram_tensor("out", [nloc, 4], f32, kind="ExternalOutput")

    # ---------------- internal DRAM
    tab_own = [nc.dram_tensor(f"tab_own{i}", [nloc, 18], f32)
               for i in range(2)]
    tab_full = [nc.dram_tensor(f"tab_full{i}", [n_pad, 18], f32,
                               addr_space="Shared") for i in range(2)]
    num_d = [nc.dram_tensor(f"num{i}", [nloc, NR], f32) for i in range(2)]

    with tile.TileContext(nc) as tc, ExitStack() as ctx:
        const = ctx.enter_context(tc.tile_pool(name="const", bufs=1))
        keep = ctx.enter_context(tc.tile_pool(name="keep", bufs=1))
        nodef = ctx.enter_context(tc.tile_pool(name="nodef", bufs=2))
        psn = ctx.enter_context(tc.tile_pool(name="psn", bufs=2,
                                             space="PSUM"))
        psb = ctx.enter_context(tc.tile_pool(name="psb", bufs=2,
                                             space="PSUM"))
        pse = ctx.enter_context(tc.tile_pool(name="pse", bufs=2,
                                             space="PSUM"))
        edge = ctx.enter_context(tc.tile_pool(name="edge", bufs=4))
        nph = ctx.enter_context(tc.tile_pool(name="nph", bufs=1))

        # ---------- constants
        iota_i = const.tile([P, B], i32)
        nc.gpsimd.iota(iota_i[:], pattern=[[1, B]], base=0,
                       channel_multiplier=0)
        iota16 = const.tile([P, B], bf16)
        nc.vector.tensor_copy(iota16[:], iota_i[:])
        ones_row = const.tile([1, P], f32)
        nc.vector.memset(ones_row[:], 1.0)
        ones16 = const.tile([1, P], bf16)
        nc.vector.memset(ones16[:], 1.0)
        ident = const.tile([P, P], bf16)
        make_identity(nc, ident[:])

        sbw = {}
        for n, s in wnames:
            t = const.tile(s, f32, tag=f"w_{n}")
            nc.sync.dma_start(t[:], wh[n][:])
            sbw[n] = t

        he_sb = keep.tile([P, NCH, 3], f32, tag="he")
        nc.sync.dma_start(he_sb[:], he_d[:])

        def bcast_row(row_ap, n, out_dt, pool, tag):
            """[1, n] row -> [P, n] tile via PE outer product."""
            ps = psb.tile([P, n], f32, tag="psb")
            ones = ones16 if row_ap.dtype == bf16 else ones_row
            nc.tensor.matmul(ps[:], lhsT=ones[:], rhs=row_ap,
                             start=True, stop=True)
            out = pool.tile([P, n], out_dt, tag=tag)
            nc.scalar.copy(out[:], ps[:])
            return out

        b_bc = [bcast_row(sbw["c1_b"][:], 32, f32, const, "bbc0"),
                bcast_row(sbw["c2_b"][:], 32, f32, const, "bbc1")]
        wdall = bcast_row(sbw["dec_WT"][:], 128, f32, const, "wdbc")
        wd_bc = [wdall[:, 32 * k:32 * (k + 1)] for k in range(4)]
        bd_bc = bcast_row(sbw["dec_b"][:], 4, f32, const, "bdbc")

        sbw16 = {}
        for n in ("c1_W", "c1_asad", "c2_W", "c2_asad",
                  "c1_adr", "c2_adr"):
            t16 = const.tile(sbw[n].shape, bf16, tag=f"w16_{n}")
            nc.vector.tensor_copy(t16[:], sbw[n][:])
            sbw16[n] = t16

        # ---------- stage A: encoder (f-major)
        h0T = keep.tile([32, nloc], bf16, tag="hT")
        for c in range(NC_A):
            sl = slice(c * NCA, (c + 1) * NCA)
            xc = nodef.tile([5, NCA], f32, tag="xc")
            nc.sync.dma_start(xc[:], xT[:, sl])
            ps = psn.tile([P, NCA], f32, tag="psn")
            nc.tensor.matmul(ps[0:32, :NCA], lhsT=sbw["enc_W"][:],
                             rhs=xc[:], start=True, stop=True)
            nc.scalar.activation(h0T[:, sl], ps[0:32, :NCA], AF.Relu,
                                 bias=sbw["enc_b"][:], scale=1.0)

        def make_table(hT, wk, asadk, adrk):
            """f-major hidden [32, nloc] -> aug rows + node-major stag."""
            augT = keep.tile([ROW, nloc], bf16, tag="augT")
            nc.vector.memset(augT[:], 0.0)
            for c in range(NC_A):
                sl = slice(c * NCA, (c + 1) * NCA)
                ps = psn.tile([P, NCA], f32, tag="psn")
                nc.tensor.matmul(ps[0:32, :NCA], lhsT=sbw16[wk][:],
                                 rhs=hT[:, sl], start=True, stop=True)
                nc.scalar.copy(augT[0:32, sl], ps[0:32, :NCA])
                ps1 = psn.tile([P, NCA], f32, tag="psn")
                nc.tensor.matmul(ps1[0:2, :NCA], lhsT=sbw16[asadk][:],
                                 rhs=augT[0:32, sl], start=True, stop=True)
                nc.vector.tensor_copy(augT[32:34, sl], ps1[0:2, :NCA])
            # adB[k, p] = att_dst[k] for all p (outer product with ones)
            psB = psb.tile([P, P], f32, tag="psb")
            nc.tensor.matmul(psB[0:32, 0:P], lhsT=sbw16[adrk][:],
                             rhs=ones16[:], start=True, stop=True)
            adB = keep.tile([32, P], bf16, tag="adB")
            nc.scalar.copy(adB[:], psB[0:32, 0:P])
            stag = keep.tile([P, NCH, ROW], bf16, tag="stag")
            for c in range(NCH):
                ps = psn.tile([P, P], bf16, tag="psnT")
                nc.tensor.transpose(out=ps[:, 0:ROW],
                                    in_=augT[:, c * P:(c + 1) * P],
                                    identity=ident[0:ROW, 0:ROW])
                nc.scalar.copy(stag[:, c, :], ps[:, 0:ROW])
            nc.vector.memset(stag[:, :, 34], 1.0)
            return augT, adB, stag

        def publish_table(stag, conv):
            own_view = tab_own[conv][:].rearrange("(c p) r -> p c r", p=P)
            nc.sync.dma_start(own_view, stag[:, :, 0:36].bitcast(f32))
            nc.gpsimd.collective_compute(
                "AllGather", mybir.AluOpType.bypass,
                replica_groups=groups,
                ins=[tab_own[conv][:]],
                outs=[tab_full[conv][:]],
            )

        augT1, adB1, stag1 = make_table(h0T, "c1_W", "c1_asad", "c1_adr")
        publish_table(stag1, 0)

        # ---------- edge pipeline
        def edge_pass(conv, augT, adB):
            ea_d = ea1_d if conv == 0 else ea2_d
            for ch in range(nchunk):
                tsl = slice(ch * CT, (ch + 1) * CT)
                srcs = edge.tile([P, CT], i32, tag="srcs")
                nc.sync.dma_start(srcs[:], src_d[:, tsl])
                dls = edge.tile([P, CT], bf16, tag="dls")
                nc.sync.dma_start(dls[:], dl_d[:, tsl])
                ea = edge.tile([P, CT], f32, tag="ea")
                nc.sync.dma_start(ea[:], ea_d[:, tsl])
                vp = edge.tile([P, CT, 18], f32, tag="v")
                v = vp[:].bitcast(bf16)          # [P, CT, 36]
                if conv == 0 and ch < 4:         # one memset per pool buffer
                    nc.vector.memset(vp[:], 0.0)
                nc.gpsimd.indirect_dma_start(
                    out=vp[:], out_offset=None,
                    in_=tab_full[conv][:],
                    in_offset=bass.IndirectOffsetOnAxis(ap=srcs[:], axis=0),
                    bounds_check=n_pad - 1, oob_is_err=False)
                oh = edge.tile([P, CT, B], bf16, tag="oh")
                nc.vector.tensor_tensor(
                    out=oh[:], in0=dls[:].to_broadcast([P, CT, B]),
                    in1=iota16[:, None, :].to_broadcast([P, CT, B]),
                    op=AT.is_equal)
                psA = psb.tile([P, CN], f32, tag="psb")
                nc.tensor.matmul(psA[:], lhsT=adB[:],
                                 rhs=augT[0:32, ch * CN:(ch + 1) * CN],
                                 start=True, stop=True)
                a32 = edge.tile([P, CN], bf16, tag="a32")
                nc.scalar.copy(a32[:], psA[:])
                dprod = edge.tile([P, CT, B], bf16, tag="dprod")
                a32v = a32[:].rearrange("p (g b) -> p g b", b=B)
                nc.vector.tensor_tensor(
                    out=dprod[:].rearrange("p (g c) b -> p g c b", c=cap),
                    in0=oh[:].rearrange("p (g c) b -> p g c b", c=cap),
                    in1=a32v[:, :, None, :].to_broadcast([P, CG, cap, B]),
                    op=AT.mult)
                dexp = edge.tile([P, CT], f32, tag="dexp")
                nc.vector.tensor_reduce(out=dexp[:], in_=dprod[:],
                                        axis=AX.X, op=AT.add)
                z = edge.tile([P, CT], f32, tag="z")
                nc.vector.tensor_copy(z[:], v[:, :, 32])   # a_src[src]
                nc.vector.tensor_tensor(out=z[:], in0=z[:], in1=ea[:],
                                        op=AT.add)
                nc.vector.tensor_tensor(out=z[:], in0=z[:], in1=dexp[:],
                                        op=AT.add)
                nc.vector.tensor_scalar_min(z[:], z[:], ZCLAMP)
                zn = edge.tile([P, CT], f32, tag="zn")
                nc.vector.tensor_scalar_mul(zn[:], z[:], NEG)
                nc.vector.tensor_tensor(out=z[:], in0=z[:], in1=zn[:],
                                        op=AT.max)
                ex = edge.tile([P, CT], bf16, tag="ex")
                nc.scalar.activation(ex[:], z[:], AF.Exp)
                ohex = edge.tile([P, CT, B], bf16, tag="ohex")
                nc.vector.tensor_tensor(
                    out=ohex[:], in0=oh[:],
                    in1=ex[:, :, None].to_broadcast([P, CT, B]),
                    op=AT.mult)
                ps = pse.tile([B, CG * NR], f32, tag="pse")
                for g in range(CG):
                    for i in range(cap):
                        t = g * cap + i
                        nc.tensor.matmul(
                            ps[:, g * NR:(g + 1) * NR],
                            lhsT=ohex[:, t, :], rhs=v[:, t, 0:NR],
                            start=(i == 0), stop=(i == cap - 1))
                st = edge.tile([B, CG * NR], f32, tag="st")
                nc.scalar.copy(st[:], ps[:])
                nc.sync.dma_start(
                    num_d[conv][ch * CN:(ch + 1) * CN]
                    .rearrange("(g b) r -> b g r", b=B),
                    st[:].rearrange("b (g r) -> b g r", r=NR))

        edge_pass(0, augT1, adB1)

        # ---------- node phase (sliced per pair of edge chunks so it
        # overlaps the edge-pass tail; 2*CN nodes = NSL node-chunks)
        NSL = 2 * CN // P

        def node_finish(conv, stag, out_relu):
            h = nph.tile([P, NCH, 32], f32, tag="h")
            for s in range(nchunk // 2):
                csl = slice(s * NSL, (s + 1) * NSL)
                num = nodef.tile([P, NSL, NR], f32, tag="num")
                nc.sync.dma_start(
                    num[:],
                    num_d[conv][s * NSL * P:(s + 1) * NSL * P]
                    .rearrange("(c p) r -> p c r", p=P))
                dg = nodef.tile([P, NSL], f32, tag="dg")
                nc.vector.tensor_scalar_max(dg[:], he_sb[:, csl, 2], 1.0)
                nc.vector.reciprocal(dg[:], dg[:])
                zl = nodef.tile([P, NSL], f32, tag="zl2")
                nc.vector.tensor_tensor(out=zl[:], in0=he_sb[:, csl, conv],
                                        in1=dg[:], op=AT.mult)
                asf = nodef.tile([P, NSL], f32, tag="asf")
                nc.vector.tensor_copy(asf[:], stag[:, csl, 32])
                nc.vector.tensor_tensor(out=zl[:], in0=zl[:], in1=asf[:],
                                        op=AT.add)
                nc.vector.tensor_copy(asf[:], stag[:, csl, 33])
                nc.vector.tensor_tensor(out=zl[:], in0=zl[:], in1=asf[:],
                                        op=AT.add)
                zln = nodef.tile([P, NSL], f32, tag="zln")
                nc.vector.tensor_scalar_mul(zln[:], zl[:], NEG)
                nc.vector.tensor_tensor(out=zl[:], in0=zl[:], in1=zln[:],
                                        op=AT.max)
                exl = nodef.tile([P, NSL], f32, tag="exl")
                nc.scalar.activation(exl[:], zl[:], AF.Exp)
                den = nodef.tile([P, NSL], f32, tag="den")
                nc.vector.tensor_tensor(out=den[:], in0=num[:, :, 34],
                                        in1=exl[:], op=AT.add)
                nc.vector.reciprocal(den[:], den[:])
                exl16 = nodef.tile([P, NSL], bf16, tag="exl16")
                nc.vector.tensor_copy(exl16[:], exl[:])
                hs = h[:, csl, :]
                nc.vector.tensor_tensor(
                    out=hs, in0=stag[:, csl, 0:32],
                    in1=exl16[:, :, None].to_broadcast([P, NSL, 32]),
                    op=AT.mult)
                nc.vector.tensor_tensor(out=hs, in0=hs,
                                        in1=num[:, :, 0:32], op=AT.add)
                nc.vector.tensor_tensor(
                    out=hs, in0=hs,
                    in1=den[:, :, None].to_broadcast([P, NSL, 32]),
                    op=AT.mult)
                nc.vector.tensor_tensor(
                    out=hs, in0=hs,
                    in1=b_bc[conv][:, None, :].to_broadcast([P, NSL, 32]),
                    op=AT.add)
                if out_relu:
                    nc.vector.tensor_scalar_max(hs, hs, 0.0)
            return h

        h1 = node_finish(0, stag1, True)

        # node-major -> f-major via PE transpose chunks
        h1T = keep.tile([32, nloc], bf16, tag="hT")
        h1b = nph.tile([P, NCH, 32], bf16, tag="h1b")
        for s in range(nchunk // 2):
            csl = slice(s * NSL, (s + 1) * NSL)
            nc.vector.tensor_copy(h1b[:, csl, :], h1[:, csl, :])
        for c in range(NCH):
            ps = psn.tile([P, P], bf16, tag="psnT")
            nc.tensor.transpose(out=ps[0:32, 0:P], in_=h1b[:, c, :],
                                identity=ident[:])
            nc.scalar.copy(h1T[:, c * P:(c + 1) * P], ps[0:32, 0:P])

        augT2, adB2, stag2 = make_table(h1T, "c2_W", "c2_asad", "c2_adr")
        publish_table(stag2, 1)
        edge_pass(1, augT2, adB2)
        h2 = node_finish(1, stag2, False)

        # ---------- decoder + log_softmax (node-major, sliced to cascade
        # behind node_finish(1) slices)
        for s in range(nchunk // 2):
            csl = slice(s * NSL, (s + 1) * NSL)
            lg = nodef.tile([P, NSL, 4], f32, tag="lg")
            tmp = nodef.tile([P, NSL, 32], f32, tag="dtmp")
            for k in range(4):
                nc.vector.tensor_tensor(
                    out=tmp[:], in0=h2[:, csl, :],
                    in1=wd_bc[k][:, None, :].to_broadcast([P, NSL, 32]),
                    op=AT.mult)
                nc.vector.tensor_reduce(out=lg[:, :, k], in_=tmp[:],
                                        axis=AX.X, op=AT.add)
            nc.vector.tensor_tensor(
                out=lg[:], in0=lg[:],
                in1=bd_bc[:, None, 0:4].to_broadcast([P, NSL, 4]), op=AT.add)
            mx = nodef.tile([P, NSL], f32, tag="mx")
            nc.vector.tensor_reduce(out=mx[:], in_=lg[:], axis=AX.X,
                                    op=AT.max)
            nc.vector.tensor_tensor(
                out=lg[:], in0=lg[:],
                in1=mx[:, :, None].to_broadcast([P, NSL, 4]),
                op=AT.subtract)
            el = nodef.tile([P, NSL, 4], f32, tag="el")
            nc.scalar.activation(el[:], lg[:], AF.Exp)
            se = nodef.tile([P, NSL], f32, tag="se")
            nc.vector.tensor_reduce(out=se[:], in_=el[:], axis=AX.X,
                                    op=AT.add)
            ls = nodef.tile([P, NSL], f32, tag="ls")
            nc.scalar.activation(ls[:], se[:], AF.Ln)
            nc.vector.tensor_tensor(
                out=lg[:], in0=lg[:],
                in1=ls[:, :, None].to_broadcast([P, NSL, 4]),
                op=AT.subtract)
            nc.sync.dma_start(
                out_d[s * NSL * P:(s + 1) * NSL * P]
                .rearrange("(c p) r -> p c r", p=P), lg[:])

    nc.compile()
    return nc


_PROGRAM_CACHE = {}


def _get_program(cfg):
    key = (cfg["nloc"], cfg["cap"])
    if key not in _PROGRAM_CACHE:
        _PROGRAM_CACHE[key] = _build_program(cfg)
    return _PROGRAM_CACHE[key]


def _prep(inputs):
    """Host preprocessing: layout + per-core input maps."""
    import ml_dtypes
    f32 = np.float32
    ei = np.asarray(inputs["edge_index"])
    attr = np.asarray(inputs["edge_attr"], f32)
    n_nodes = np.asarray(inputs["x"]).shape[0]
    n_cores = 8
    src = ei[0].astype(np.int32)
    dst = ei[1].astype(np.int32)

    w1 = (np.asarray(inputs["c1_We"], f32)
          @ np.asarray(inputs["c1_att_e"], f32))
    w2 = (np.asarray(inputs["c2_We"], f32)
          @ np.asarray(inputs["c2_att_e"], f32))
    ea1 = (attr @ w1).astype(f32)
    ea2 = (attr @ w2).astype(f32)

    cfg, src_t, dstl_t, dl_t, e1_t, e2_t, he_t, newidx = _build_layout(
        src, dst, ea1, ea2, n_nodes, n_cores)

    x = np.asarray(inputs["x"], f32)
    nloc, n_pad = cfg["nloc"], cfg["n_pad"]
    xp = np.zeros((n_pad, 5), f32)
    xp[newidx] = x
    cfg["newidx"] = newidx
    com = {
        "enc_W": np.asarray(inputs["enc_W"], f32),
        "enc_b": np.asarray(inputs["enc_b"], f32).reshape(32, 1),
        "c1_W": np.asarray(inputs["c1_W"], f32),
        "c1_asad": np.stack([np.asarray(inputs["c1_att_src"], f32),
                             np.asarray(inputs["c1_att_dst"], f32)], axis=1),
        "c1_b": np.asarray(inputs["c1_b"], f32).reshape(1, 32),
        "c2_W": np.asarray(inputs["c2_W"], f32),
        "c2_asad": np.stack([np.asarray(inputs["c2_att_src"], f32),
                             np.asarray(inputs["c2_att_dst"], f32)], axis=1),
        "c2_b": np.asarray(inputs["c2_b"], f32).reshape(1, 32),
        "dec_WT": np.asarray(inputs["dec_W"], f32).T.copy().reshape(1, 128),
        "dec_b": np.asarray(inputs["dec_b"], f32).reshape(1, 4),
        "c1_adr": np.asarray(inputs["c1_att_dst"], f32).reshape(1, 32),
        "c2_adr": np.asarray(inputs["c2_att_dst"], f32).reshape(1, 32),
    }
    in_maps = []
    for c in range(n_cores):
        m = dict(com)
        m["xT"] = xp[c * nloc:(c + 1) * nloc].T.copy()
        m["src"] = src_t[c]
        m["dstl"] = dstl_t[c]
        m["dl"] = dl_t[c].astype(ml_dtypes.bfloat16)
        m["ea1"] = e1_t[c]
        m["ea2"] = e2_t[c]
        m["he"] = he_t[c]
        in_maps.append(m)
    return cfg, in_maps


# ------------------------------------------------------------------ entrypoint
def kernel(**inputs):
    n_trucks = int(inputs["num_trucks"])
    cfg, in_maps = _prep(inputs)
    n_cores = cfg["n_cores"]

    nc = _get_program(cfg)
    from concourse.bass_utils import run_bass_kernel_spmd
    res = run_bass_kernel_spmd(nc, in_maps, core_ids=list(range(n_cores)),
                               trace=False)
    outs = [res.results[c]["out"] for c in range(n_cores)]
    full = np.concatenate(outs, axis=0)[cfg["newidx"][:n_trucks]]
    return np.asarray(full, np.float32)
